# revision 11
# baseline (speedup 1.0000x reference)
"""Causal multi-head attention (B=4, S=2048, D=1024, H=16) on 8 TRN2 NeuronCores.

Sharding: DP=4 over batch x TP=2 over heads (8 heads per core). Host sums the
two TP partials per batch and adds bo.

v2 design (ACT-bound pipeline, ~all exp time hidden behind nothing):
  - The scalar engine (exp over 21M score elements/core at 1 elem/cycle/lane
    @1.2GHz) is the serial floor (~180us). Everything else is scheduled to
    hide under it.
  - h-merged score groups: both heads of a pair write ONE [128,1024] PSUM
    tile (h0 cols 0:512 = bank A, h1 cols 512:1024 = bank B). One exp op
    covers both heads; the two score matmuls (K=64, lhsT base partitions
    0/64 -> auto row-tiling) become ready together, sit adjacent in the PE
    queue, and execute concurrently in the PE array halves.
  - Streaming lead-in: xk/xq/xv loaded in 512-col quarter blocks, wk/wq in
    per-pair column blocks, emitted in strict need-order on two DMA queues;
    K/Q projection and attention of pair 0 start ~11us in.
  - PE + exp-table warmup during the DMA lead-in (HAM clock gate + ACT
    table load prepaid).
  - On-chip softmax normalization: V carries a ones column per head so PV
    yields denominators in PSUM row 64; DVE reciprocal -> [2,512] row tile;
    a K=2 matmul against a {0,1} selector broadcasts the two reciprocal
    rows across the 128 partitions (h0 -> 0:63, h1 -> 64:127); one fused
    tensor_mul per head normalizes + drains PSUM -> atp bf16. No DRAM
    bounce, no partition-broadcast DMAs.
  - Filler injection: V projection (VN 4..15), the next pair's K/Q
    projection, and (for the last pair) per-j out-projection chunks are
    injected one-per-k-iteration into the attention loop, filling the PE
    slack under the exp stream. The kernel ends ~1 normalization + 8
    matmul-chains after the last exp instead of a 50us serialized tail.
"""

import sys

sys.path.insert(0, "/opt/trn_rl_repo")

import numpy as np

B = 4
S = 2048
D = 1024
H = 16
HD = 64
TP = 2
DH = D // TP          # 512 head-dims per core (8 heads)
NHL = DH // HD        # 8 local heads
DCH = 4               # pairs: d-chunks of 128 within DH
NKT = S // 128        # 16 key tiles
NQT = S // 512        # 4 query tiles of 512
KCH = D // 128        # 8 contraction chunks for projections

_compiled = None


def _build():
    import concourse.bacc as bacc
    import concourse.mybir as mybir
    import concourse.tile as tile

    F32 = mybir.dt.float32
    BF16 = mybir.dt.bfloat16
    EXP = mybir.ActivationFunctionType.Exp

    nc = bacc.Bacc("TRN2", target_bir_lowering=False, debug=False)

    xq = nc.dram_tensor("xq", [D, S], BF16, kind="ExternalInput")
    xk = nc.dram_tensor("xk", [D, S], BF16, kind="ExternalInput")
    xv = nc.dram_tensor("xv", [D, S], BF16, kind="ExternalInput")
    wq = nc.dram_tensor("wq", [DCH, D, 128], BF16, kind="ExternalInput")
    wk = nc.dram_tensor("wk", [DCH, D, 128], BF16, kind="ExternalInput")
    wv = nc.dram_tensor("wv", [D, DH], BF16, kind="ExternalInput")
    wo = nc.dram_tensor("wo", [DH, D], BF16, kind="ExternalInput")
    bq_c = nc.dram_tensor("bq_c", [128, DCH], F32, kind="ExternalInput")
    bk_c = nc.dram_tensor("bk_c", [128, DCH], F32, kind="ExternalInput")
    bv_b = nc.dram_tensor("bv_b", [128, DH], F32, kind="ExternalInput")
    out = nc.dram_tensor("out", [S, D], F32, kind="ExternalOutput")

    with tile.TileContext(nc) as tc:
        with (
            tc.tile_pool(name="cst", bufs=1) as cst,
            tc.tile_pool(name="xb", bufs=1) as xb,
            tc.tile_pool(name="wp", bufs=1) as wp,
            tc.tile_pool(name="kq", bufs=1) as kqp,
            tc.tile_pool(name="vn", bufs=1) as vn_pool,
            tc.tile_pool(name="atp", bufs=1) as atp_pool,
            tc.tile_pool(name="pr", bufs=4) as pr_pool,
            tc.tile_pool(name="nrm", bufs=2) as nrm_pool,
            tc.tile_pool(name="ob", bufs=2) as ob_pool,
            tc.tile_pool(name="psS", bufs=2, space="PSUM") as psS,
            tc.tile_pool(name="psV", bufs=2, space="PSUM") as psV,
        ):
            # ---------------- constants / small tiles ----------------
            scratch = cst.tile([128, 512], BF16, tag="scr", name="scratch")
            nc.gpsimd.memset(scratch[:, :], 0.0)

            # causal mask base [128, 896]: mask[x, c] = 1.0 iff c - x >= 384.
            # diagonal k-tile i (0..3) of a 512-q tile uses slice
            # mask[:, 384-128i : 896-128i]  ->  valid iff q >= kpos + 128 i.
            mask = cst.tile([128, 896], BF16, tag="mask", name="mask")
            nc.gpsimd.memset(mask[:, :], 1.0)
            nc.gpsimd.affine_select(
                out=mask[:, :],
                in_=mask[:, :],
                compare_op=mybir.AluOpType.is_ge,
                fill=0.0,
                base=-384,
                pattern=[[1, 896]],
                channel_multiplier=-1,
            )

            ones = cst.tile([128, NHL], F32, tag="ones", name="ones")
            nc.vector.memset(ones[:, :], 1.0)

            # ones rows for the reciprocal broadcast matmuls (rows 0 and 32
            # are used as K=1 stationary operands).
            onesw = cst.tile([64, 64], BF16, tag="onesw", name="onesw")
            nc.vector.memset(onesw[:, :], 1.0)

            bqs = cst.tile([128, DCH], F32, tag="bqs", name="bqs")
            bks = cst.tile([128, DCH], F32, tag="bks", name="bks")
            bvb = cst.tile([128, DH], F32, tag="bvb", name="bvb")

            # ---------------- persistent big tiles ----------------
            xkb = xb.tile([128, KCH * S], BF16, tag="xkb", name="xkb")
            xqb = xb.tile([128, KCH * S], BF16, tag="xqb", name="xqb")
            xvb = xb.tile([128, KCH * S], BF16, tag="xvb", name="xvb")
            wkb = wp.tile([128, KCH * DH], BF16, tag="wkb", name="wkb")
            wqb = wp.tile([128, KCH * DH], BF16, tag="wqb", name="wqb")
            # wvb is reused for wo after the 16 V-projection chunks have
            # consumed it (KCH*DH == DCH*D == 4096); the wo DMA is emitted
            # as a filler after VN15 so WAR deps sequence it automatically.
            wvb = wp.tile([128, KCH * DH], BF16, tag="wvb", name="wvb")
            wob = wvb

            KT = [kqp.tile([128, S], BF16, tag=f"kt{d}", name=f"KT{d}")
                  for d in range(DCH)]
            QT = [kqp.tile([128, S], BF16, tag=f"qt{d}", name=f"QT{d}")
                  for d in range(DCH)]
            # V natural [seq, 8*(64+1)]: head h cols 65h..65h+63, ones at
            # 65h+64 (PV then emits the softmax denominator in psum row 64)
            VN = [vn_pool.tile([128, NHL * (HD + 1)], BF16, tag=f"vn{i}",
                               name=f"VN{i}")
                  for i in range(NKT)]
            for v in VN:
                nc.vector.tensor_copy(v[:, HD::HD + 1], ones[:, :])
            atp = [atp_pool.tile([128, S], BF16, tag=f"atp{d}",
                                 name=f"atp{d}")
                   for d in range(DCH)]

            # ---------------- DMA emission (strict need-order) --------
            def xq_dma(big, src, sc):
                dst = big[:, :].rearrange("r (k c) -> r k c", c=S)[
                    :, :, 512 * sc:512 * (sc + 1)]
                s = src[:, 512 * sc:512 * (sc + 1)].rearrange(
                    "(k r) c -> r k c", r=128)
                return dst, s

            def w_dma(big, src, p):
                dst = big[:, :].rearrange("r (k c) -> r k c", c=DH)[
                    :, :, 128 * p:128 * (p + 1)]
                s = src[p].rearrange("(k r) c -> r k c", r=128)
                return dst, s

            # sync queue
            nc.sync.dma_start(out=bks[:, :], in_=bk_c[:, :])
            nc.sync.dma_start(out=bqs[:, :], in_=bq_c[:, :])
            nc.sync.dma_start(out=bvb[:, :], in_=bv_b[:, :])
            d, s = xq_dma(xkb, xk, 0)
            nc.sync.dma_start(out=d, in_=s)
            nc.sync.dma_start(
                out=wvb[:, :].rearrange("r (k c) -> r k c", c=DH),
                in_=wv.rearrange("(k r) c -> r k c", r=128))
            for sc in (1, 2):
                d, s = xq_dma(xkb, xk, sc)
                nc.sync.dma_start(out=d, in_=s)
            d, s = xq_dma(xvb, xv, 2)
            nc.sync.dma_start(out=d, in_=s)
            d, s = xq_dma(xkb, xk, 3)
            nc.sync.dma_start(out=d, in_=s)
            for p in (2,):
                d, s = w_dma(wkb, wk, p)
                nc.sync.dma_start(out=d, in_=s)
                d, s = w_dma(wqb, wq, p)
                nc.sync.dma_start(out=d, in_=s)

            def wo_dma():
                nc.sync.dma_start(
                    out=wob[:, :].rearrange("r (c d) -> r c d", d=D),
                    in_=wo.rearrange("(c r) d -> r c d", r=128))

            # gpsimd queue
            d, s = w_dma(wkb, wk, 0)
            nc.gpsimd.dma_start(out=d, in_=s)
            d, s = w_dma(wqb, wq, 0)
            nc.gpsimd.dma_start(out=d, in_=s)
            d, s = xq_dma(xqb, xq, 0)
            nc.gpsimd.dma_start(out=d, in_=s)
            d, s = xq_dma(xvb, xv, 0)
            nc.gpsimd.dma_start(out=d, in_=s)
            d, s = xq_dma(xqb, xq, 1)
            nc.gpsimd.dma_start(out=d, in_=s)
            d, s = xq_dma(xvb, xv, 1)
            nc.gpsimd.dma_start(out=d, in_=s)
            d, s = xq_dma(xqb, xq, 2)
            nc.gpsimd.dma_start(out=d, in_=s)
            for p in (1,):
                d, s = w_dma(wkb, wk, p)
                nc.gpsimd.dma_start(out=d, in_=s)
                d, s = w_dma(wqb, wq, p)
                nc.gpsimd.dma_start(out=d, in_=s)
            d, s = xq_dma(xqb, xq, 3)
            nc.gpsimd.dma_start(out=d, in_=s)
            d, s = xq_dma(xvb, xv, 3)
            nc.gpsimd.dma_start(out=d, in_=s)
            for p in (3,):
                d, s = w_dma(wkb, wk, p)
                nc.gpsimd.dma_start(out=d, in_=s)
                d, s = w_dma(wqb, wq, p)
                nc.gpsimd.dma_start(out=d, in_=s)

            # ---------------- warmups ----------------
            # ACT exp-table load prepaid on a tiny op.
            nc.scalar.activation(scratch[0:1, 64:128],
                                 scratch[0:1, 0:64], EXP, scale=1.0)
            # PE warmup: ~12 back-to-back N=512 matmuls (~4us at the cold
            # clock) flips the HAM clock gate before real work arrives.
            for w in range(12):
                ps = psS.tile([128, 1024], F32, tag="ps", name="warm_")
                nc.tensor.matmul(ps[:, 0:512], scratch[:, 0:128],
                                 scratch[:, 0:512], start=True, stop=True)

            # ---------------- building blocks ----------------
            def kq_proj_chunk(p, which, sc):
                wt, dest, bias = ((wkb, KT, bks) if which == "k"
                                  else (wqb, QT, bqs))
                ps = psS.tile([128, 1024], F32, tag="ps", name="psKQ_")
                for ki in range(KCH):
                    nc.tensor.matmul(
                        ps[:, 0:512],
                        wt[:, DH * ki + 128 * p:DH * ki + 128 * (p + 1)],
                        xkb[:, S * ki + 512 * sc:S * ki + 512 * (sc + 1)]
                        if which == "k" else
                        xqb[:, S * ki + 512 * sc:S * ki + 512 * (sc + 1)],
                        start=(ki == 0),
                        stop=(ki == KCH - 1),
                    )
                nc.vector.tensor_scalar_add(
                    dest[p][:, 512 * sc:512 * (sc + 1)],
                    ps[:, 0:512],
                    bias[:, p:p + 1],
                )

            def v_proj_chunk(st):
                ps = psS.tile([128, 1024], F32, tag="ps", name="psv_")
                for ki in range(KCH):
                    nc.tensor.matmul(
                        ps[:, 0:512],
                        xvb[:, S * ki + 128 * st:S * ki + 128 * (st + 1)],
                        wvb[:, DH * ki:DH * (ki + 1)],
                        start=(ki == 0),
                        stop=(ki == KCH - 1),
                    )
                vdst = VN[st][:, :].rearrange(
                    "p (h c) -> p h c", c=HD + 1)[:, :, :HD]
                nc.vector.tensor_add(
                    vdst,
                    ps[:, 0:512].rearrange("p (h c) -> p h c", c=HD),
                    bvb[:, :].rearrange("p (h c) -> p h c", c=HD),
                )

            def outproj_chunk(qt, n):
                ps = psS.tile([128, 1024], F32, tag="ps", name="psO_")
                for c in range(DCH):
                    nc.tensor.matmul(
                        ps[:, 0:512],
                        atp[c][:, 128 * qt:128 * (qt + 1)],
                        wob[:, D * c + 512 * n:D * c + 512 * (n + 1)],
                        start=(c == 0),
                        stop=(c == DCH - 1),
                    )
                ot = ob_pool.tile([128, 512], F32, tag="ob", name="ob_")
                nc.vector.tensor_copy(ot[:, :], ps[:, 0:512])
                nc.sync.dma_start(
                    out=out[128 * qt:128 * (qt + 1),
                            512 * n:512 * (n + 1)],
                    in_=ot[:, :])

            fillers = []

            def inject():
                if fillers:
                    fillers.pop(0)()

            # ---------------- pair 0 lead-in ----------------
            for sc in range(NQT):
                kq_proj_chunk(0, "k", sc)
                kq_proj_chunk(0, "q", sc)
            for st in range(4):
                v_proj_chunk(st)

            # ---------------- attention p-loop ----------------
            for p in range(DCH):
                for j in range(NQT):
                    # Fillers are paced so a filler's input DMA is already
                    # resident (or nearly) when it reaches the in-order PE
                    # queue: VN(st) projections go one xv-quarter behind
                    # the lead-in stream; the next pair's K/Q projection
                    # and the wo load land in j=3.
                    if p == 0 and j >= 1:
                        for st in range(4 * j, 4 * (j + 1)):
                            fillers.append(
                                lambda st=st: v_proj_chunk(st))
                        if j == 3:
                            fillers.append(wo_dma)
                    if j == 3 and p < DCH - 1:
                        for sc in range(NQT):
                            fillers.append(
                                lambda sc=sc, p=p: kq_proj_chunk(
                                    p + 1, "k", sc))
                            fillers.append(
                                lambda sc=sc, p=p: kq_proj_chunk(
                                    p + 1, "q", sc))
                    q0 = 512 * j
                    nk = 4 * (j + 1)
                    pv = [psV.tile([HD + 1, 512], F32, tag=f"pv{h}",
                                   name=f"pv{h}_")
                          for h in range(2)]
                    for k in range(nk):
                        pss = psS.tile([128, 1024], F32, tag="ps",
                                       name="psS_")
                        for h in range(2):
                            r0 = 64 * h
                            nc.tensor.matmul(
                                pss[:, 512 * h:512 * (h + 1)],
                                KT[p][r0:r0 + 64, 128 * k:128 * (k + 1)],
                                QT[p][r0:r0 + 64, q0:q0 + 512],
                                start=True,
                                stop=True,
                            )
                        prt = pr_pool.tile([128, 1024], BF16, tag="pr",
                                           name="pr_")
                        nc.scalar.activation(prt[:, :], pss[:, :], EXP,
                                             scale=0.125)
                        i = k - 4 * j
                        if 0 <= i <= 3:
                            msl = mask[:, 384 - 128 * i:896 - 128 * i]
                            for h in range(2):
                                nc.vector.tensor_mul(
                                    prt[:, 512 * h:512 * (h + 1)],
                                    prt[:, 512 * h:512 * (h + 1)],
                                    msl,
                                )
                        for h in range(2):
                            hl = 2 * p + h
                            nc.tensor.matmul(
                                pv[h][:, :],
                                VN[k][:, 65 * hl:65 * hl + 65],
                                prt[:, 512 * h:512 * (h + 1)],
                                start=(k == 0),
                                stop=(k == nk - 1),
                            )
                        inject()

                    # normalization for (p, j): emitted as a filler so the
                    # PE-side broadcast matmul lands after the next j's
                    # first score pair (hides the reciprocal latency).
                    def norm(p=p, j=j, pv=pv):
                        q0 = 512 * j
                        rc = nrm_pool.tile([33, 512], BF16, tag="rc",
                                           name="rc_")
                        with nc.allow_low_precision(
                                reason="denominator reciprocal in bf16"):
                            nc.vector.reciprocal(rc[0:1, :],
                                                 pv[0][HD:HD + 1, :])
                            nc.vector.reciprocal(rc[32:33, :],
                                                 pv[1][HD:HD + 1, :])
                        bct = psS.tile([128, 1024], F32, tag="ps",
                                       name="psB_")
                        nc.tensor.matmul(bct[0:64, 0:512],
                                         onesw[0:1, :], rc[0:1, :],
                                         start=True, stop=True)
                        nc.tensor.matmul(bct[64:128, 0:512],
                                         onesw[32:33, :], rc[32:33, :],
                                         start=True, stop=True)
                        bcs = nrm_pool.tile([128, 512], BF16, tag="bcs",
                                            name="bcs_")
                        nc.vector.tensor_copy(bcs[:, :], bct[:, 0:512])
                        for h in range(2):
                            nc.vector.tensor_mul(
                                atp[p][64 * h:64 * (h + 1), q0:q0 + 512],
                                pv[h][0:HD, :],
                                bcs[64 * h:64 * (h + 1), :],
                            )

                    last = (p == DCH - 1 and j == NQT - 1)
                    if last:
                        norm()
                        for qt in range(4 * j, 4 * (j + 1)):
                            for n in range(2):
                                outproj_chunk(qt, n)
                    else:
                        fillers.append(norm)
                        if p == DCH - 1:
                            for qt in range(4 * j, 4 * (j + 1)):
                                for n in range(2):
                                    fillers.append(
                                        lambda qt=qt, n=n: outproj_chunk(
                                            qt, n))
                # drain pending fillers before the next pair's scores
                # (KT/QT of p+1 must be emitted before they are read)
                if p < DCH - 1:
                    while fillers:
                        fillers.pop(0)()
            while fillers:
                fillers.pop(0)()

    nc.compile()
    return nc


def kernel(query, key, value, Wq, bq, Wk, bk, Wv, bv, Wo, bo, **trace_kwargs):
    from concourse.bass_utils import run_bass_kernel_spmd

    global _compiled
    if _compiled is None:
        _compiled = _build()
    nc = _compiled

    import ml_dtypes

    BF = ml_dtypes.bfloat16
    query = np.asarray(query, np.float32)
    key = np.asarray(key, np.float32)
    value = np.asarray(value, np.float32)
    Wq, Wk, Wv, Wo = (np.asarray(w, np.float32) for w in (Wq, Wk, Wv, Wo))
    bq, bk, bv, bo = (np.asarray(b_, np.float32) for b_ in (bq, bk, bv, bo))

    xqT = [np.ascontiguousarray(query[b].T).astype(BF) for b in range(B)]
    xkT = [np.ascontiguousarray(key[b].T).astype(BF) for b in range(B)]
    xvT = [np.ascontiguousarray(value[b].T).astype(BF) for b in range(B)]

    def pack_w(Wm, cs):
        # [D, 512] -> [4, D, 128] pair-major contiguous
        return np.ascontiguousarray(
            Wm[:, cs].reshape(D, DCH, 128).transpose(1, 0, 2)).astype(BF)

    shard = []
    for t in range(TP):
        cs = slice(DH * t, DH * (t + 1))
        shard.append({
            "wq": pack_w(Wq, cs),
            "wk": pack_w(Wk, cs),
            "wv": np.ascontiguousarray(Wv[:, cs]).astype(BF),
            "wo": np.ascontiguousarray(Wo[cs, :]).astype(BF),
            "bq_c": np.ascontiguousarray(bq[cs].reshape(DCH, 128).T),
            "bk_c": np.ascontiguousarray(bk[cs].reshape(DCH, 128).T),
            "bv_b": np.ascontiguousarray(
                np.broadcast_to(bv[cs], (128, DH))),
        })

    in_maps = []
    for c in range(8):
        b, t = c // TP, c % TP
        m = {"xq": xqT[b], "xk": xkT[b], "xv": xvT[b]}
        m.update(shard[t])
        in_maps.append(m)

    res = run_bass_kernel_spmd(nc, in_maps, core_ids=list(range(8)),
                               **trace_kwargs)
    outp = np.empty((B, S, D), np.float32)
    for b in range(B):
        outp[b] = res.results[TP * b]["out"] + res.results[TP * b + 1]["out"] + bo
    if trace_kwargs:
        kernel.last_results = res
    return outp


# revision 19
# speedup vs baseline: 1.0081x; 1.0081x over previous
"""Causal multi-head attention (B=4, S=2048, D=1024, H=16) on 8 TRN2 NeuronCores.

Sharding: DP=4 over batch x TP=2 over heads (8 heads per core). Host sums the
two TP partials per batch and adds bo.

v2 design (ACT-bound pipeline, ~all exp time hidden behind nothing):
  - The scalar engine (exp over 21M score elements/core at 1 elem/cycle/lane
    @1.2GHz) is the serial floor (~180us). Everything else is scheduled to
    hide under it.
  - h-merged score groups: both heads of a pair write ONE [128,1024] PSUM
    tile (h0 cols 0:512 = bank A, h1 cols 512:1024 = bank B). One exp op
    covers both heads; the two score matmuls (K=64, lhsT base partitions
    0/64 -> auto row-tiling) become ready together, sit adjacent in the PE
    queue, and execute concurrently in the PE array halves.
  - Streaming lead-in: xk/xq/xv loaded in 512-col quarter blocks, wk/wq in
    per-pair column blocks, emitted in strict need-order on two DMA queues;
    K/Q projection and attention of pair 0 start ~11us in.
  - PE + exp-table warmup during the DMA lead-in (HAM clock gate + ACT
    table load prepaid).
  - On-chip softmax normalization: V carries a ones column per head so PV
    yields denominators in PSUM row 64; DVE reciprocal -> [2,512] row tile;
    a K=2 matmul against a {0,1} selector broadcasts the two reciprocal
    rows across the 128 partitions (h0 -> 0:63, h1 -> 64:127); one fused
    tensor_mul per head normalizes + drains PSUM -> atp bf16. No DRAM
    bounce, no partition-broadcast DMAs.
  - Filler injection: V projection (VN 4..15), the next pair's K/Q
    projection, and (for the last pair) per-j out-projection chunks are
    injected one-per-k-iteration into the attention loop, filling the PE
    slack under the exp stream. The kernel ends ~1 normalization + 8
    matmul-chains after the last exp instead of a 50us serialized tail.
"""

import sys

sys.path.insert(0, "/opt/trn_rl_repo")

import numpy as np

B = 4
S = 2048
D = 1024
H = 16
HD = 64
TP = 2
DH = D // TP          # 512 head-dims per core (8 heads)
NHL = DH // HD        # 8 local heads
DCH = 4               # pairs: d-chunks of 128 within DH
NKT = S // 128        # 16 key tiles
NQT = S // 512        # 4 query tiles of 512
KCH = D // 128        # 8 contraction chunks for projections

_compiled = None


def _build():
    import concourse.bacc as bacc
    import concourse.mybir as mybir
    import concourse.tile as tile

    F32 = mybir.dt.float32
    BF16 = mybir.dt.bfloat16
    EXP = mybir.ActivationFunctionType.Exp

    nc = bacc.Bacc("TRN2", target_bir_lowering=False, debug=False)

    # x tensors are host-pre-blocked [sc][ki][128][512] so each
    # (sc, ki) 128KB block is linear in DRAM -> simple [128, 512] DMA APs.
    xq = nc.dram_tensor("xq", [NQT, KCH, 128, 512], BF16,
                        kind="ExternalInput")
    xk = nc.dram_tensor("xk", [NQT, KCH, 128, 512], BF16,
                        kind="ExternalInput")
    xv = nc.dram_tensor("xv", [NQT, KCH, 128, 512], BF16,
                        kind="ExternalInput")
    # weights host-transposed to partition-major: single contiguous DMAs.
    wq = nc.dram_tensor("wq", [DCH, 128, KCH * 128], BF16,
                        kind="ExternalInput")
    wk = nc.dram_tensor("wk", [DCH, 128, KCH * 128], BF16,
                        kind="ExternalInput")
    wv = nc.dram_tensor("wv", [128, KCH * DH], BF16, kind="ExternalInput")
    wo = nc.dram_tensor("wo", [128, DCH * D], BF16, kind="ExternalInput")
    bq_c = nc.dram_tensor("bq_c", [128, DCH], F32, kind="ExternalInput")
    bk_c = nc.dram_tensor("bk_c", [128, DCH], F32, kind="ExternalInput")
    bv_b = nc.dram_tensor("bv_b", [128, DH], F32, kind="ExternalInput")
    out = nc.dram_tensor("out", [S, D], F32, kind="ExternalOutput")

    with tile.TileContext(nc) as tc:
        with (
            tc.tile_pool(name="cst", bufs=1) as cst,
            tc.tile_pool(name="xb", bufs=1) as xb,
            tc.tile_pool(name="wp", bufs=1) as wp,
            tc.tile_pool(name="kq", bufs=1) as kqp,
            tc.tile_pool(name="vn", bufs=1) as vn_pool,
            tc.tile_pool(name="atp", bufs=1) as atp_pool,
            tc.tile_pool(name="pr", bufs=4) as pr_pool,
            tc.tile_pool(name="nrm", bufs=2) as nrm_pool,
            tc.tile_pool(name="ob", bufs=2) as ob_pool,
            tc.tile_pool(name="psS", bufs=2, space="PSUM") as psS,
            tc.tile_pool(name="psV", bufs=2, space="PSUM") as psV,
        ):
            # ---------------- constants / small tiles ----------------
            scratch = cst.tile([128, 512], BF16, tag="scr", name="scratch")
            nc.gpsimd.memset(scratch[:, :], 0.0)

            # causal mask base [128, 896]: mask[x, c] = 1.0 iff c - x >= 384.
            # diagonal k-tile i (0..3) of a 512-q tile uses slice
            # mask[:, 384-128i : 896-128i]  ->  valid iff q >= kpos + 128 i.
            mask = cst.tile([128, 896], BF16, tag="mask", name="mask")
            nc.gpsimd.memset(mask[:, :], 1.0)
            nc.gpsimd.affine_select(
                out=mask[:, :],
                in_=mask[:, :],
                compare_op=mybir.AluOpType.is_ge,
                fill=0.0,
                base=-384,
                pattern=[[1, 896]],
                channel_multiplier=-1,
            )

            ones = cst.tile([128, NHL], F32, tag="ones", name="ones")
            nc.vector.memset(ones[:, :], 1.0)

            # ones rows for the reciprocal broadcast matmuls (rows 0 and 32
            # are used as K=1 stationary operands).
            onesw = cst.tile([64, 64], BF16, tag="onesw", name="onesw")
            nc.vector.memset(onesw[:, :], 1.0)
            # fp32 staging rows for reciprocal_approx_fast (rows 0 and 32
            # live; memset once so whole-tile casts read initialized data)
            rf = cst.tile([33, 512], F32, tag="rf", name="rf")
            nc.vector.memset(rf[:, :], 1.0)

            bqs = cst.tile([128, DCH], F32, tag="bqs", name="bqs")
            bks = cst.tile([128, DCH], F32, tag="bks", name="bks")
            bvb = cst.tile([128, DH], F32, tag="bvb", name="bvb")

            # ---------------- persistent big tiles ----------------
            xkb = xb.tile([128, KCH * S], BF16, tag="xkb", name="xkb")
            xqb = xb.tile([128, KCH * S], BF16, tag="xqb", name="xqb")
            xvb = xb.tile([128, KCH * S], BF16, tag="xvb", name="xvb")
            wkb = wp.tile([128, KCH * DH], BF16, tag="wkb", name="wkb")
            wqb = wp.tile([128, KCH * DH], BF16, tag="wqb", name="wqb")
            # wvb is reused for wo after the 16 V-projection chunks have
            # consumed it (KCH*DH == DCH*D == 4096); the wo DMA is emitted
            # as a filler after VN15 so WAR deps sequence it automatically.
            wvb = wp.tile([128, KCH * DH], BF16, tag="wvb", name="wvb")
            wob = wvb

            KT = [kqp.tile([128, S], BF16, tag=f"kt{d}", name=f"KT{d}")
                  for d in range(DCH)]
            QT = [kqp.tile([128, S], BF16, tag=f"qt{d}", name=f"QT{d}")
                  for d in range(DCH)]
            # V natural [seq, 8*(64+1)]: head h cols 65h..65h+63, ones at
            # 65h+64 (PV then emits the softmax denominator in psum row 64)
            VN = [vn_pool.tile([128, NHL * (HD + 1)], BF16, tag=f"vn{i}",
                               name=f"VN{i}")
                  for i in range(NKT)]
            for v in VN:
                nc.vector.tensor_copy(v[:, HD::HD + 1], ones[:, :])
            atp = [atp_pool.tile([128, S], BF16, tag=f"atp{d}",
                                 name=f"atp{d}")
                   for d in range(DCH)]

            # ---------------- DMA emission (strict need-order) --------
            # x quarter (sc, ki): [128, 512] block, linear in DRAM.
            def xdma(eng, big, src, sc):
                for ki in range(KCH):
                    eng.dma_start(
                        out=big[:, S * ki + 512 * sc:
                                S * ki + 512 * (sc + 1)],
                        in_=src[sc, ki])

            def wdma(eng, big, src, p):
                eng.dma_start(out=big[:, 1024 * p:1024 * (p + 1)],
                              in_=src[p])

            # sync queue (need-order)
            nc.sync.dma_start(out=bks[:, :], in_=bk_c[:, :])
            nc.sync.dma_start(out=bqs[:, :], in_=bq_c[:, :])
            nc.sync.dma_start(out=bvb[:, :], in_=bv_b[:, :])
            wdma(nc.sync, wkb, wk, 0)
            xdma(nc.sync, xkb, xk, 0)
            nc.sync.dma_start(out=wvb[:, :], in_=wv[:, :])
            xdma(nc.sync, xkb, xk, 1)
            xdma(nc.sync, xvb, xv, 1)
            xdma(nc.sync, xkb, xk, 2)
            xdma(nc.sync, xvb, xv, 2)
            wdma(nc.sync, wkb, wk, 2)
            wdma(nc.sync, wqb, wq, 2)

            def wo_dma():
                nc.sync.dma_start(out=wob[:, :], in_=wo[:, :])

            # gpsimd queue (need-order)
            wdma(nc.gpsimd, wqb, wq, 0)
            xdma(nc.gpsimd, xqb, xq, 0)
            xdma(nc.gpsimd, xvb, xv, 0)
            xdma(nc.gpsimd, xqb, xq, 1)
            xdma(nc.gpsimd, xqb, xq, 2)
            wdma(nc.gpsimd, wkb, wk, 1)
            wdma(nc.gpsimd, wqb, wq, 1)
            xdma(nc.gpsimd, xkb, xk, 3)
            xdma(nc.gpsimd, xqb, xq, 3)
            xdma(nc.gpsimd, xvb, xv, 3)
            wdma(nc.gpsimd, wkb, wk, 3)
            wdma(nc.gpsimd, wqb, wq, 3)

            # ---------------- warmups ----------------
            # ACT exp-table load prepaid on a tiny op.
            nc.scalar.activation(scratch[0:1, 64:128],
                                 scratch[0:1, 0:64], EXP, scale=1.0)
            # PE warmup: ~12 back-to-back N=512 matmuls (~4us at the cold
            # clock) flips the HAM clock gate before real work arrives.
            for w in range(12):
                ps = psS.tile([128, 1024], F32, tag="ps", name="warm_")
                nc.tensor.matmul(ps[:, 0:512], scratch[:, 0:128],
                                 scratch[:, 0:512], start=True, stop=True)

            # ---------------- building blocks ----------------
            def kq_proj_chunk(p, which, sc):
                wt, dest, bias = ((wkb, KT, bks) if which == "k"
                                  else (wqb, QT, bqs))
                xt = xkb if which == "k" else xqb
                ps = psS.tile([128, 1024], F32, tag="ps", name="psKQ_")
                for ki in range(KCH):
                    nc.tensor.matmul(
                        ps[:, 0:512],
                        wt[:, 1024 * p + 128 * ki:
                           1024 * p + 128 * (ki + 1)],
                        xt[:, S * ki + 512 * sc:S * ki + 512 * (sc + 1)],
                        start=(ki == 0),
                        stop=(ki == KCH - 1),
                    )
                nc.vector.tensor_scalar_add(
                    dest[p][:, 512 * sc:512 * (sc + 1)],
                    ps[:, 0:512],
                    bias[:, p:p + 1],
                )

            def v_proj_chunk(st):
                ps = psS.tile([128, 1024], F32, tag="ps", name="psv_")
                for ki in range(KCH):
                    nc.tensor.matmul(
                        ps[:, 0:512],
                        xvb[:, S * ki + 128 * st:S * ki + 128 * (st + 1)],
                        wvb[:, DH * ki:DH * (ki + 1)],
                        start=(ki == 0),
                        stop=(ki == KCH - 1),
                    )
                vdst = VN[st][:, :].rearrange(
                    "p (h c) -> p h c", c=HD + 1)[:, :, :HD]
                nc.vector.tensor_add(
                    vdst,
                    ps[:, 0:512].rearrange("p (h c) -> p h c", c=HD),
                    bvb[:, :].rearrange("p (h c) -> p h c", c=HD),
                )

            def outproj_chunk(qt, n):
                ps = psS.tile([128, 1024], F32, tag="ps", name="psO_")
                for c in range(DCH):
                    nc.tensor.matmul(
                        ps[:, 0:512],
                        atp[c][:, 128 * qt:128 * (qt + 1)],
                        wob[:, D * c + 512 * n:D * c + 512 * (n + 1)],
                        start=(c == 0),
                        stop=(c == DCH - 1),
                    )
                ot = ob_pool.tile([128, 512], F32, tag="ob", name="ob_")
                nc.vector.tensor_copy(ot[:, :], ps[:, 0:512])
                nc.sync.dma_start(
                    out=out[128 * qt:128 * (qt + 1),
                            512 * n:512 * (n + 1)],
                    in_=ot[:, :])

            fillers = []

            def inject():
                if fillers:
                    fillers.pop(0)()

            # ---------------- pair 0 lead-in ----------------
            for sc in range(NQT):
                kq_proj_chunk(0, "k", sc)
                kq_proj_chunk(0, "q", sc)
            for st in range(4):
                v_proj_chunk(st)

            # ---------------- attention p-loop ----------------
            for p in range(DCH):
                for j in range(NQT):
                    # Fillers are paced so a filler's input DMA is already
                    # resident (or nearly) when it reaches the in-order PE
                    # queue: VN(st) projections go one xv-quarter behind
                    # the lead-in stream; the next pair's K/Q projection
                    # and the wo load land in j=3.
                    if p == 0 and j >= 1:
                        for st in range(4 * j, 4 * (j + 1)):
                            fillers.append(
                                lambda st=st: v_proj_chunk(st))
                        if j == 3:
                            fillers.append(wo_dma)
                    if j == 3 and p < DCH - 1:
                        for sc in range(NQT):
                            fillers.append(
                                lambda sc=sc, p=p: kq_proj_chunk(
                                    p + 1, "k", sc))
                            fillers.append(
                                lambda sc=sc, p=p: kq_proj_chunk(
                                    p + 1, "q", sc))
                    q0 = 512 * j
                    nk = 4 * (j + 1)
                    pv = [psV.tile([HD + 1, 512], F32, tag=f"pv{h}",
                                   name=f"pv{h}_")
                          for h in range(2)]
                    for k in range(nk):
                        pss = psS.tile([128, 1024], F32, tag="ps",
                                       name="psS_")
                        for h in range(2):
                            r0 = 64 * h
                            nc.tensor.matmul(
                                pss[:, 512 * h:512 * (h + 1)],
                                KT[p][r0:r0 + 64, 128 * k:128 * (k + 1)],
                                QT[p][r0:r0 + 64, q0:q0 + 512],
                                start=True,
                                stop=True,
                            )
                        prt = pr_pool.tile([128, 1024], BF16, tag="pr",
                                           name="pr_")
                        nc.scalar.activation(prt[:, :], pss[:, :], EXP,
                                             scale=0.125)
                        i = k - 4 * j
                        if 0 <= i <= 3:
                            msl = mask[:, 384 - 128 * i:896 - 128 * i]
                            for h in range(2):
                                nc.vector.tensor_mul(
                                    prt[:, 512 * h:512 * (h + 1)],
                                    prt[:, 512 * h:512 * (h + 1)],
                                    msl,
                                )
                        for h in range(2):
                            hl = 2 * p + h
                            nc.tensor.matmul(
                                pv[h][:, :],
                                VN[k][:, 65 * hl:65 * hl + 65],
                                prt[:, 512 * h:512 * (h + 1)],
                                start=(k == 0),
                                stop=(k == nk - 1),
                            )
                        inject()

                    # normalization for (p, j): emitted as a filler so the
                    # PE-side broadcast matmul lands after the next j's
                    # first score pair (hides the reciprocal latency).
                    def norm(p=p, j=j, pv=pv):
                        q0 = 512 * j
                        nc.vector.reciprocal(rf[0:1, :],
                                             pv[0][HD:HD + 1, :])
                        nc.vector.reciprocal(rf[32:33, :],
                                             pv[1][HD:HD + 1, :])
                        rc = nrm_pool.tile([33, 512], BF16, tag="rc",
                                           name="rc_")
                        nc.vector.tensor_copy(rc[:, :], rf[:, :])
                        bct = psS.tile([128, 1024], F32, tag="ps",
                                       name="psB_")
                        nc.tensor.matmul(bct[0:64, 0:512],
                                         onesw[0:1, :], rc[0:1, :],
                                         start=True, stop=True)
                        nc.tensor.matmul(bct[64:128, 0:512],
                                         onesw[32:33, :], rc[32:33, :],
                                         start=True, stop=True)
                        bcs = nrm_pool.tile([128, 512], BF16, tag="bcs",
                                            name="bcs_")
                        nc.vector.tensor_copy(bcs[:, :], bct[:, 0:512])
                        for h in range(2):
                            nc.vector.tensor_mul(
                                atp[p][64 * h:64 * (h + 1), q0:q0 + 512],
                                pv[h][0:HD, :],
                                bcs[64 * h:64 * (h + 1), :],
                            )

                    last = (p == DCH - 1 and j == NQT - 1)
                    if last:
                        norm()
                        for qt in range(4 * j, 4 * (j + 1)):
                            for n in range(2):
                                outproj_chunk(qt, n)
                    else:
                        fillers.append(norm)
                        if p == DCH - 1:
                            for qt in range(4 * j, 4 * (j + 1)):
                                for n in range(2):
                                    fillers.append(
                                        lambda qt=qt, n=n: outproj_chunk(
                                            qt, n))
                # drain pending fillers before the next pair's scores
                # (KT/QT of p+1 must be emitted before they are read)
                if p < DCH - 1:
                    while fillers:
                        fillers.pop(0)()
            while fillers:
                fillers.pop(0)()

    nc.compile()
    return nc


def kernel(query, key, value, Wq, bq, Wk, bk, Wv, bv, Wo, bo, **trace_kwargs):
    from concourse.bass_utils import run_bass_kernel_spmd

    global _compiled
    if _compiled is None:
        _compiled = _build()
    nc = _compiled

    import ml_dtypes

    BF = ml_dtypes.bfloat16
    query = np.asarray(query, np.float32)
    key = np.asarray(key, np.float32)
    value = np.asarray(value, np.float32)
    Wq, Wk, Wv, Wo = (np.asarray(w, np.float32) for w in (Wq, Wk, Wv, Wo))
    bq, bk, bv, bo = (np.asarray(b_, np.float32) for b_ in (bq, bk, bv, bo))

    def pack_x(xb):
        # x[b].T is [D, S]; block to [sc][ki][128][512] (linear quarters)
        xT = xb.T.reshape(KCH, 128, NQT, 512)
        return np.ascontiguousarray(xT.transpose(2, 0, 1, 3)).astype(BF)

    xqT = [pack_x(query[b]) for b in range(B)]
    xkT = [pack_x(key[b]) for b in range(B)]
    xvT = [pack_x(value[b]) for b in range(B)]

    def pack_w(Wm, cs):
        # [D, 512] -> [4 pairs][128 part][8 ki x 128] contiguous
        w = Wm[:, cs].reshape(KCH, 128, DCH, 128)
        return np.ascontiguousarray(
            w.transpose(2, 1, 0, 3).reshape(DCH, 128, KCH * 128)).astype(BF)

    def pack_kmaj(Wm):
        # [D, 512] -> [128 part][8 ki x 512] (ki-major cols)
        w = Wm.reshape(KCH, 128, DH)
        return np.ascontiguousarray(
            w.transpose(1, 0, 2).reshape(128, KCH * DH)).astype(BF)

    def pack_wo(Wm):
        # [512, D] -> [128 part][4 c x 1024]
        w = Wm.reshape(DCH, 128, D)
        return np.ascontiguousarray(
            w.transpose(1, 0, 2).reshape(128, DCH * D)).astype(BF)

    shard = []
    for t in range(TP):
        cs = slice(DH * t, DH * (t + 1))
        shard.append({
            "wq": pack_w(Wq, cs),
            "wk": pack_w(Wk, cs),
            "wv": pack_kmaj(Wv[:, cs]),
            "wo": pack_wo(Wo[cs, :]),
            "bq_c": np.ascontiguousarray(bq[cs].reshape(DCH, 128).T),
            "bk_c": np.ascontiguousarray(bk[cs].reshape(DCH, 128).T),
            "bv_b": np.ascontiguousarray(
                np.broadcast_to(bv[cs], (128, DH))),
        })

    in_maps = []
    for c in range(8):
        b, t = c // TP, c % TP
        m = {"xq": xqT[b], "xk": xkT[b], "xv": xvT[b]}
        m.update(shard[t])
        in_maps.append(m)

    res = run_bass_kernel_spmd(nc, in_maps, core_ids=list(range(8)),
                               **trace_kwargs)
    outp = np.empty((B, S, D), np.float32)
    for b in range(B):
        outp[b] = res.results[TP * b]["out"] + res.results[TP * b + 1]["out"] + bo
    if trace_kwargs:
        kernel.last_results = res
    return outp
